# revision 1
# baseline (speedup 1.0000x reference)
"""Trainium2 Bass kernel for nn_CustomLinear (rewired linear layer).

The reference computes  out = x @ W.T + bias  plus a per-output-row "rewire"
correction: for rewire e on row r with src column s and clone columns d_k
(denom = K+1), x[:, s] and x[:, d_k] are all replaced by x[:, s]/denom before
the GEMV with weight[r].  Because the correction is linear in x with
coefficients built from the original W, it folds exactly into a modified
weight matrix W':

    dW[r, s]   += (1/denom - 1) * W[r, s] + (1/denom) * sum_k W[r, d_k]
    dW[r, d_k] += -W[r, d_k]
    out = x @ (W + dW).T + bias            (exact, duplicates accumulate)

So the device-side work is one dense GEMM.  Sharding: data-parallel over the
flattened batch axis N across 8 NeuronCores (4096 rows each); W', bias
replicated.  Per core the GEMM runs in bf16 (fp32 PSUM accumulation) at the
PE's full rate; x is pre-packed on the host into a blocked-transposed layout
[32, 128(j), 8(ko), 128(n)] so each x-tile load is one contiguous 256KB DMA
and no on-chip transposes are needed.
"""

import numpy as np
import ml_dtypes

import concourse.bass as bass  # noqa: F401  (bass must import before tile)
import concourse.tile as tile
import concourse.mybir as mybir
from concourse import bacc
from concourse.bass_utils import run_bass_kernel_spmd

N_CORES = 8
N = 32768
IN_F = 1024
OUT_F = 1024
P = 128
NS = N // N_CORES          # 4096 rows per core
MT = NS // P               # 32 m-tiles per core
KO = IN_F // P             # 8 k-subtiles
OC = 512                   # PSUM free-dim chunk (one fp32 bank)

_nc_cache = None


def _build_nc():
    global _nc_cache
    if _nc_cache is not None:
        return _nc_cache

    nc = bacc.Bacc("TRN2", target_bir_lowering=False, debug=False)
    xb_d = nc.dram_tensor("xb", [MT, P, KO, P], mybir.dt.bfloat16,
                          kind="ExternalInput")
    wt_d = nc.dram_tensor("wt", [KO, P, OUT_F], mybir.dt.bfloat16,
                          kind="ExternalInput")
    bias_d = nc.dram_tensor("bias", [P, OUT_F], mybir.dt.float32,
                            kind="ExternalInput")
    out_d = nc.dram_tensor("out", [NS, OUT_F], mybir.dt.float32,
                           kind="ExternalOutput")

    with tile.TileContext(nc) as tc:
        with (
            tc.tile_pool(name="wpool", bufs=1) as wpool,
            tc.tile_pool(name="xpool", bufs=3) as xpool,
            tc.tile_pool(name="opool", bufs=3) as opool,
            tc.tile_pool(name="pspool", bufs=4, space="PSUM") as pspool,
        ):
            # Resident weights [128, 8, 1024] bf16 (16KB/partition) + bias.
            wt_sb = wpool.tile([P, KO, OUT_F], mybir.dt.bfloat16, tag="wt")
            nc.sync.dma_start(wt_sb[:], wt_d.ap().rearrange("k p o -> p k o"))
            bias_sb = wpool.tile([P, OUT_F], mybir.dt.float32, tag="bias")
            nc.sync.dma_start(bias_sb[:], bias_d.ap())

            for m in range(MT):
                xt = xpool.tile([P, KO, P], mybir.dt.bfloat16, tag="xt")
                nc.sync.dma_start(xt[:], xb_d.ap()[m])

                out_sb = opool.tile([P, OUT_F], mybir.dt.float32, tag="osb")
                for oc in range(OUT_F // OC):
                    ps = pspool.tile([P, OC], mybir.dt.float32, tag="ps")
                    for ko in range(KO):
                        nc.tensor.matmul(
                            ps[:],
                            xt[:, ko, :],
                            wt_sb[:, ko, oc * OC:(oc + 1) * OC],
                            start=(ko == 0),
                            stop=(ko == KO - 1),
                        )
                    nc.vector.tensor_add(
                        out_sb[:, oc * OC:(oc + 1) * OC], ps[:],
                        bias_sb[:, oc * OC:(oc + 1) * OC],
                    )
                nc.sync.dma_start(out_d.ap()[m * P:(m + 1) * P, :], out_sb[:])

    nc.compile()
    _nc_cache = nc
    return nc


def _fold_rewires(weight, rewire_rows, rewire_src, rewire_clones):
    """Fold the rewire corrections into the weight matrix (exact, fp32)."""
    r = np.asarray(rewire_rows, dtype=np.int64)
    s = np.asarray(rewire_src, dtype=np.int64)
    d = np.asarray(rewire_clones, dtype=np.int64)
    denom = d.shape[1] + 1
    w_rs = weight[r, s]                      # [R]
    w_rd = weight[r[:, None], d]             # [R, K]
    dW = np.zeros_like(weight)
    np.add.at(dW, (r, s), (1.0 / denom - 1.0) * w_rs + w_rd.sum(axis=1) / denom)
    np.add.at(dW, (r[:, None], d), -w_rd)
    return weight + dW


def kernel(x, weight, bias, rewire_rows, rewire_src, rewire_clones):
    x = np.asarray(x)
    weight = np.asarray(weight, dtype=np.float32)
    bias = np.asarray(bias, dtype=np.float32)

    wp = _fold_rewires(weight, rewire_rows, rewire_src, rewire_clones)
    # W'^T in [ko, p(j), o] blocks, bf16.
    wt = np.ascontiguousarray(wp.T).astype(ml_dtypes.bfloat16)
    wt = wt.reshape(KO, P, OUT_F)
    bias128 = np.ascontiguousarray(np.broadcast_to(bias, (P, OUT_F)))

    # Pack x: per core [4096, 1024] -> [32, 128(j), 8(ko), 128(n)] bf16.
    xb16 = np.asarray(x, dtype=np.float32).astype(ml_dtypes.bfloat16)
    in_maps = []
    for c in range(N_CORES):
        xs = xb16[c * NS:(c + 1) * NS]
        xbl = np.ascontiguousarray(
            xs.reshape(MT, P, KO, P).transpose(0, 3, 2, 1))
        in_maps.append({"xb": xbl, "wt": wt, "bias": bias128})

    nc = _build_nc()
    res = run_bass_kernel_spmd(nc, in_maps, list(range(N_CORES)))
    out = np.concatenate([res.results[c]["out"] for c in range(N_CORES)],
                         axis=0)
    return np.ascontiguousarray(out, dtype=np.float32)
